# revision 19
# baseline (speedup 1.0000x reference)
"""Word-encoder masked-attention pooling (segment softmax-reduce) on 8 trn2 cores.

Sharding: n_words split across 8 cores (750 words each).  Spans are sorted
and contiguous.  Word tiles (128 words) are packed in PAIRS: both tiles of a
pair share one gathered row window (saves ~17% of row blocks vs per-tile
gathering); a row block straddling both tiles simply feeds two matmul sets.

Host staging (all invertible repacking):
  hg[p, t*1056 + c] : row block t (128 gathered hidden rows, bf16), each
      hidden column c pre-scaled by w_attn[c]; col 1025 = 1.0 (denominator
      matmul operand); 64B-aligned pitch.
  mk[p, s*128 + w]  : 0/1 span mask for matmul-set s (row block x word tile)
Device, per row block t:
  s   = row_sum(hg_block[:, 0:1024])     (= h.w;  DVE reduce / ACT accum)
  e   = Exp(s)                           (ACT, [128,1])
per matmul set (block t, word tile m):
  mke = mask_s * e_t                     (gpsimd tensor_scalar, [128,128])
  pn_m += mke^T @ hg_t[:, 0:1024]        (PE bf16, two 512-wide matmuls)
  pd_m += mke^T @ hg_t[:, 1025]          (PE bf16, 1-wide matmul)
per word tile: r = 1/pd (DVE, 1 tile late), out = pn*r -> bf16 (ACT, 2 tiles
late so it never blocks a later tile in the ACT stream), DMA out.
Host unscales out column c by 1/w_attn[c] and upcasts to f32.
b_attn is constant so it cancels in the softmax.
"""

import ml_dtypes
import numpy as np
from contextlib import ExitStack

import concourse.bass as bass
import concourse.bacc as bacc
import concourse.mybir as mybir
import concourse.tile as tile
from concourse.bass_utils import run_bass_kernel_spmd

NCORES = 8
P = 128
HID = 1024
HB = 1056  # block pitch: 1024 scaled cols + spare + ones col + pad (64B align)
ONES_COL = HID + 1

LAST_RESULT = None  # BassKernelResults of the most recent run (for profiling)

_prog_cache = {}

# row blocks whose score row-sum runs on ACT (Copy+accum) instead of DVE
# (mid-stream: late blocks would clog ACT right when final tiles need it)
ACT_SCORE = (4, 6)


def _build_program(geom):
    """One SPMD program for all cores.

    geom = (MT, KBp, tile_ks) with KBp[g] row blocks for pair g and
    tile_ks[m] = (k0, k1) the inclusive relative block range tile m consumes.
    """
    MT, KBp, tile_ks = geom
    T = sum(KBp)
    base = [0]
    for g in range(len(KBp) - 1):
        base.append(base[-1] + KBp[g])
    S = sum(k1 - k0 + 1 for k0, k1 in tile_ks)
    f32 = mybir.dt.float32
    bf16 = mybir.dt.bfloat16
    fp8 = mybir.dt.float8e4
    EXP = mybir.ActivationFunctionType.Exp
    COPY = mybir.ActivationFunctionType.Copy
    nc = bacc.Bacc(
        "TRN2", target_bir_lowering=False, debug=False, num_devices=NCORES
    )
    hg = nc.declare_dram_parameter("hg", [P, T * HB], bf16, isOutput=False)
    mk = nc.declare_dram_parameter("mk", [P, S * P], fp8, isOutput=False)
    out = nc.declare_dram_parameter("out", [P, MT * HID], bf16, isOutput=True)

    with tile.TileContext(nc) as tc, ExitStack() as ctx:
        cpool = ctx.enter_context(tc.tile_pool(name="c", bufs=1))
        hpool = ctx.enter_context(tc.tile_pool(name="h", bufs=len(KBp) + 1))
        mpool = ctx.enter_context(tc.tile_pool(name="m", bufs=1))
        mepool = ctx.enter_context(tc.tile_pool(name="me", bufs=6))
        scpool = ctx.enter_context(tc.tile_pool(name="sc", bufs=2))
        spool = ctx.enter_context(tc.tile_pool(name="s", bufs=10))
        rpool = ctx.enter_context(tc.tile_pool(name="r", bufs=3))
        opool = ctx.enter_context(tc.tile_pool(name="o", bufs=3))
        pnpool = ctx.enter_context(tc.tile_pool(name="pn", bufs=3, space="PSUM"))
        pdpool = ctx.enter_context(tc.tile_pool(name="pd", bufs=2, space="PSUM"))

        # warm the ACT exp table off the critical path
        dummy = cpool.tile([P, 1], f32)
        nc.vector.memset(dummy[:], 0.0)
        nc.scalar.activation(dummy[:], dummy[:], EXP)

        # h blocks on the sync ring, one DMA per block; the mask slab rides
        # the same ring after the first two blocks (so block 0 lands first,
        # uncontended, and masks still arrive before the first mke)
        htiles = []
        mkt = mpool.tile([P, S * P], fp8)
        issued = 0
        for g in range(len(KBp)):
            ht = hpool.tile([P, KBp[g] * HB], bf16)
            for kb in range(KBp[g]):
                nc.sync.dma_start(
                    ht[:, kb * HB : (kb + 1) * HB],
                    hg[:, (base[g] + kb) * HB : (base[g] + kb + 1) * HB],
                )
                issued += 1
                if issued == 2:
                    nc.gpsimd.dma_start(mkt[:], mk[:, :])
            htiles.append(ht)

        def emit_scores(g, ht, nkb):
            # per-block row-sums into columns of one tile, then a single
            # batched exp for the whole pair (ACT cost is fixed-overhead
            # dominated at this width)
            sp = spool.tile([P, nkb], f32)
            for kb in range(nkb):
                t = base[g] + kb
                if t in ACT_SCORE:
                    scratch = scpool.tile([P, HID], bf16)
                    nc.scalar.activation(
                        scratch[:], ht[:, kb * HB : kb * HB + HID], COPY,
                        accum_out=sp[:, kb : kb + 1],
                    )
                else:
                    nc.vector.tensor_reduce(
                        sp[:, kb : kb + 1],
                        ht[:, kb * HB : kb * HB + HID],
                        mybir.AxisListType.X,
                        mybir.AluOpType.add,
                    )
            ep = spool.tile([P, nkb], f32)
            nc.scalar.activation(ep[:], sp[:], EXP)
            return ep

        def emit_recip(m, pn, pd):
            r = rpool.tile([P, 1], f32)
            nc.vector.reciprocal(r[:], pd[:, 0:1])
            return r

        def emit_out(m, pn, r):
            o = opool.tile([P, HID], bf16)
            nc.scalar.activation(o[:], pn[:], COPY, scale=r[:])
            eng = nc.sync if m % 2 == 0 else nc.gpsimd
            eng.dma_start(out[:, m * HID : (m + 1) * HID], o[:])

        s_idx = 0
        q = []  # [m, pn, pd, r] tails awaiting emission, oldest first
        for g in range(len(KBp)):
            ht = htiles[g]
            ep = emit_scores(g, ht, KBp[g])
            for m in (2 * g, 2 * g + 1):
                if m >= MT:
                    continue
                k0, k1 = tile_ks[m]
                pn = pnpool.tile([P, HID], f32)
                pd = pdpool.tile([P, 8], f32)
                for kb in range(k0, k1 + 1):
                    mke = mepool.tile([P, P], bf16)
                    nc.gpsimd.tensor_scalar(
                        out=mke[:],
                        in0=mkt[:, s_idx * P : (s_idx + 1) * P],
                        scalar1=ep[:, kb : kb + 1],
                        scalar2=0.0,
                        op0=mybir.AluOpType.mult,
                        op1=mybir.AluOpType.add,
                    )
                    s_idx += 1
                    first, last = kb == k0, kb == k1
                    nc.tensor.matmul(
                        pd[:, 0:1],
                        mke[:],
                        ht[:, kb * HB + ONES_COL : kb * HB + ONES_COL + 1],
                        start=first,
                        stop=last,
                    )
                    for half in range(2):
                        cs = slice(
                            kb * HB + half * 512, kb * HB + (half + 1) * 512
                        )
                        nc.tensor.matmul(
                            pn[:, half * 512 : (half + 1) * 512],
                            mke[:],
                            ht[:, cs],
                            start=first,
                            stop=last,
                        )

                q.append([m, pn, pd, None])
                if len(q) >= 2:
                    q[-2][3] = emit_recip(q[-2][0], q[-2][1], q[-2][2])
                if len(q) >= 3:
                    mm, pnn, _, rr = q.pop(0)
                    emit_out(mm, pnn, rr)
        q[-1][3] = emit_recip(q[-1][0], q[-1][1], q[-1][2])
        for mm, pnn, _, rr in q:
            emit_out(mm, pnn, rr)

    nc.compile()
    return nc


def kernel(hidden_states, word_starts, word_ends, w_attn, b_attn):
    global LAST_RESULT
    H = np.asarray(hidden_states, dtype=np.float32)
    ws = np.asarray(word_starts).astype(np.int64)
    we = np.asarray(word_ends).astype(np.int64)
    wv = np.asarray(w_attn, dtype=np.float32).reshape(-1)
    ns, hid = H.shape
    nw = ws.shape[0]
    assert hid == HID
    # pre-scale hidden columns by w (bf16); device output is unscaled by host
    wb = wv.astype(ml_dtypes.bfloat16).astype(np.float32)
    assert np.abs(wb).min() > 1e-30
    Hw = (H * wb[None, :]).astype(ml_dtypes.bfloat16)
    Wpc = (nw + NCORES - 1) // NCORES  # words per core
    MT = (Wpc + P - 1) // P  # word-tiles per core
    NG = (MT + 1) // 2  # tile pairs per core

    def tile_bounds(c, m):
        lo = c * Wpc + m * P
        hi = min(lo + P, (c + 1) * Wpc, nw)
        return lo, hi

    def pair_lo(c, g):
        return tile_bounds(c, 2 * g)[0]

    # pair row-block counts and per-tile relative block ranges (max/min over
    # cores so the SPMD program is uniform; masks zero out inactive parts)
    KBp = []
    tile_ks = []
    for g in range(NG):
        kb = 1
        for c in range(NCORES):
            lo = pair_lo(c, g)
            hi = tile_bounds(c, min(2 * g + 1, MT - 1))[1]
            R = int(we[lo:hi].max() - ws[lo] + 1)
            kb = max(kb, (R + P - 1) // P)
        KBp.append(kb)
    for m in range(MT):
        g = m // 2
        k0, k1 = 10**9, -1
        for c in range(NCORES):
            plo = pair_lo(c, g)
            lo, hi = tile_bounds(c, m)
            if lo >= hi:
                continue
            k0 = min(k0, (int(ws[lo]) - int(ws[plo])) // P)
            k1 = max(k1, (int(we[lo:hi].max()) - int(ws[plo])) // P)
        tile_ks.append((k0, k1))
    T = sum(KBp)
    S = sum(k1 - k0 + 1 for k0, k1 in tile_ks)
    base = [0]
    for g in range(NG - 1):
        base.append(base[-1] + KBp[g])

    pos = np.arange(P, dtype=np.int64)
    in_maps = []
    for c in range(NCORES):
        Hg = np.zeros((P, T * HB), ml_dtypes.bfloat16)
        Mk = np.zeros((P, S * P), ml_dtypes.float8_e4m3)
        for g in range(NG):
            kstart = int(ws[pair_lo(c, g)])
            for k in range(KBp[g]):
                j = kstart + k * P + pos  # global rows of this block
                valid = j < ns
                blk = Hg[:, (base[g] + k) * HB : (base[g] + k + 1) * HB]
                blk[valid, :HID] = Hw[j[valid]]
                blk[:, ONES_COL] = 1.0
        s_idx = 0
        for m in range(MT):
            g = m // 2
            kstart = int(ws[pair_lo(c, g)])
            lo, hi = tile_bounds(c, m)
            a = ws[lo:hi] - kstart
            b = we[lo:hi] - kstart
            k0, k1 = tile_ks[m]
            for k in range(k0, k1 + 1):
                jr = (k * P + pos)[:, None]  # rows relative to kstart
                valid = (kstart + k * P + pos) < ns
                msk = (jr >= a[None, :]) & (jr <= b[None, :]) & valid[:, None]
                Mk[:, s_idx * P : s_idx * P + (hi - lo)] = msk.astype(
                    ml_dtypes.float8_e4m3
                )
                s_idx += 1
        in_maps.append({"hg": Hg, "mk": Mk})

    geom = (MT, tuple(KBp), tuple(tile_ks))
    nc = _prog_cache.get(geom)
    if nc is None:
        nc = _build_program(geom)
        _prog_cache[geom] = nc

    res = run_bass_kernel_spmd(nc, in_maps, list(range(NCORES)))
    LAST_RESULT = res
    inv_w = (1.0 / wb).astype(np.float32)
    parts = []
    for c in range(NCORES):
        o = np.asarray(res.results[c]["out"])  # [P, MT*HID] bf16
        o = o.reshape(P, MT, HID).transpose(1, 0, 2).reshape(MT * P, HID)
        parts.append(o[:Wpc])
    full = np.concatenate(parts, axis=0)[:nw].astype(np.float32) * inv_w[None, :]
    return np.ascontiguousarray(full)


# revision 20
# speedup vs baseline: 1.0997x; 1.0997x over previous
"""Word-encoder masked-attention pooling (segment softmax-reduce) on 8 trn2 cores.

Sharding: n_words split across 8 cores (750 words each).  Spans are sorted
and contiguous.  Word tiles (128 words) are packed in PAIRS: both tiles of a
pair share one gathered row window (saves ~17% of row blocks vs per-tile
gathering); a row block straddling both tiles simply feeds two matmul sets.

Host staging (all invertible repacking):
  hg[p, t*1056 + c] : row block t (128 gathered hidden rows, bf16), each
      hidden column c pre-scaled by w_attn[c]; col 1025 = 1.0 (denominator
      matmul operand); 64B-aligned pitch.
  mk[p, s*128 + w]  : 0/1 span mask for matmul-set s (row block x word tile)
Device, per row block t:
  s   = row_sum(hg_block[:, 0:1024])     (= h.w;  DVE reduce / ACT accum)
  e   = Exp(s)                           (ACT, [128,1])
per matmul set (block t, word tile m):
  mke = mask_s * e_t                     (gpsimd tensor_scalar, [128,128])
  pn_m += mke^T @ hg_t[:, 0:1024]        (PE bf16, two 512-wide matmuls)
  pd_m += mke^T @ hg_t[:, 1025]          (PE bf16, 1-wide matmul)
per word tile: r = 1/pd (DVE, 1 tile late), out = pn*r -> bf16 (ACT, 2 tiles
late so it never blocks a later tile in the ACT stream), DMA out.
Host unscales out column c by 1/w_attn[c] and upcasts to f32.
b_attn is constant so it cancels in the softmax.
"""

import ml_dtypes
import numpy as np
from contextlib import ExitStack

import concourse.bass as bass
import concourse.bacc as bacc
import concourse.mybir as mybir
import concourse.tile as tile
from concourse.bass_utils import run_bass_kernel_spmd

NCORES = 8
P = 128
HID = 1024
HB = 1056  # block pitch: 1024 scaled cols + spare + ones col + pad (64B align)
ONES_COL = HID + 1

LAST_RESULT = None  # BassKernelResults of the most recent run (for profiling)

_prog_cache = {}

# row blocks whose score row-sum runs on ACT (Copy+accum) instead of DVE
# (mid-stream: late blocks would clog ACT right when final tiles need it)
ACT_SCORE = (5,)


def _build_program(geom):
    """One SPMD program for all cores.

    geom = (MT, KBp, tile_ks) with KBp[g] row blocks for pair g and
    tile_ks[m] = (k0, k1) the inclusive relative block range tile m consumes.
    """
    MT, KBp, tile_ks = geom
    T = sum(KBp)
    base = [0]
    for g in range(len(KBp) - 1):
        base.append(base[-1] + KBp[g])
    S = sum(k1 - k0 + 1 for k0, k1 in tile_ks)
    f32 = mybir.dt.float32
    bf16 = mybir.dt.bfloat16
    fp8 = mybir.dt.float8e4
    EXP = mybir.ActivationFunctionType.Exp
    COPY = mybir.ActivationFunctionType.Copy
    nc = bacc.Bacc(
        "TRN2", target_bir_lowering=False, debug=False, num_devices=NCORES
    )
    hg = nc.declare_dram_parameter("hg", [P, T * HB], bf16, isOutput=False)
    mk = nc.declare_dram_parameter("mk", [P, S * P], fp8, isOutput=False)
    out = nc.declare_dram_parameter("out", [P, MT * HID], bf16, isOutput=True)

    with tile.TileContext(nc) as tc, ExitStack() as ctx:
        cpool = ctx.enter_context(tc.tile_pool(name="c", bufs=1))
        hpool = ctx.enter_context(tc.tile_pool(name="h", bufs=len(KBp) + 1))
        mpool = ctx.enter_context(tc.tile_pool(name="m", bufs=1))
        mepool = ctx.enter_context(tc.tile_pool(name="me", bufs=6))
        scpool = ctx.enter_context(tc.tile_pool(name="sc", bufs=2))
        spool = ctx.enter_context(tc.tile_pool(name="s", bufs=10))
        rpool = ctx.enter_context(tc.tile_pool(name="r", bufs=3))
        opool = ctx.enter_context(tc.tile_pool(name="o", bufs=3))
        pnpool = ctx.enter_context(tc.tile_pool(name="pn", bufs=3, space="PSUM"))
        pdpool = ctx.enter_context(tc.tile_pool(name="pd", bufs=2, space="PSUM"))

        # warm the ACT exp table off the critical path
        dummy = cpool.tile([P, 1], f32)
        nc.vector.memset(dummy[:], 0.0)
        nc.scalar.activation(dummy[:], dummy[:], EXP)

        # h blocks on the sync ring, one DMA per block; the mask slab rides
        # the same ring after the first two blocks (so block 0 lands first,
        # uncontended, and masks still arrive before the first mke)
        htiles = []
        mkt = mpool.tile([P, S * P], fp8)
        issued = 0
        for g in range(len(KBp)):
            ht = hpool.tile([P, KBp[g] * HB], bf16)
            for kb in range(KBp[g]):
                nc.sync.dma_start(
                    ht[:, kb * HB : (kb + 1) * HB],
                    hg[:, (base[g] + kb) * HB : (base[g] + kb + 1) * HB],
                )
                issued += 1
                if issued == 2:
                    nc.gpsimd.dma_start(mkt[:], mk[:, :])
            htiles.append(ht)

        def emit_score(t, ht, kb):
            s = spool.tile([P, 1], f32)
            if t in ACT_SCORE:
                scratch = scpool.tile([P, HID], bf16)
                nc.scalar.activation(
                    scratch[:], ht[:, kb * HB : kb * HB + HID], COPY,
                    accum_out=s[:],
                )
            else:
                nc.vector.tensor_reduce(
                    s[:],
                    ht[:, kb * HB : kb * HB + HID],
                    mybir.AxisListType.X,
                    mybir.AluOpType.add,
                )
            e = spool.tile([P, 1], f32)
            nc.scalar.activation(e[:], s[:], EXP)
            return e

        def emit_recip(m, pn, pd):
            r = rpool.tile([P, 1], f32)
            nc.vector.reciprocal(r[:], pd[:, 0:1])
            return r

        def emit_out(m, pn, r):
            o = opool.tile([P, HID], bf16)
            nc.scalar.activation(o[:], pn[:], COPY, scale=r[:])
            eng = nc.sync if m % 2 == 0 else nc.gpsimd
            eng.dma_start(out[:, m * HID : (m + 1) * HID], o[:])

        s_idx = 0
        q = []  # [m, pn, pd, r] tails awaiting emission, oldest first
        for g in range(len(KBp)):
            ht = htiles[g]
            evals = [emit_score(base[g] + kb, ht, kb) for kb in range(KBp[g])]
            for m in (2 * g, 2 * g + 1):
                if m >= MT:
                    continue
                k0, k1 = tile_ks[m]
                pn = pnpool.tile([P, HID], f32)
                pd = pdpool.tile([P, 8], f32)
                for kb in range(k0, k1 + 1):
                    mke = mepool.tile([P, P], bf16)
                    nc.gpsimd.tensor_scalar(
                        out=mke[:],
                        in0=mkt[:, s_idx * P : (s_idx + 1) * P],
                        scalar1=evals[kb][:],
                        scalar2=0.0,
                        op0=mybir.AluOpType.mult,
                        op1=mybir.AluOpType.add,
                    )
                    s_idx += 1
                    first, last = kb == k0, kb == k1
                    nc.tensor.matmul(
                        pd[:, 0:1],
                        mke[:],
                        ht[:, kb * HB + ONES_COL : kb * HB + ONES_COL + 1],
                        start=first,
                        stop=last,
                    )
                    for half in range(2):
                        cs = slice(
                            kb * HB + half * 512, kb * HB + (half + 1) * 512
                        )
                        nc.tensor.matmul(
                            pn[:, half * 512 : (half + 1) * 512],
                            mke[:],
                            ht[:, cs],
                            start=first,
                            stop=last,
                        )

                q.append([m, pn, pd, None])
                if len(q) >= 2:
                    q[-2][3] = emit_recip(q[-2][0], q[-2][1], q[-2][2])
                if len(q) >= 3:
                    mm, pnn, _, rr = q.pop(0)
                    emit_out(mm, pnn, rr)
        q[-1][3] = emit_recip(q[-1][0], q[-1][1], q[-1][2])
        for mm, pnn, _, rr in q:
            emit_out(mm, pnn, rr)

    nc.compile()
    return nc


def kernel(hidden_states, word_starts, word_ends, w_attn, b_attn):
    global LAST_RESULT
    H = np.asarray(hidden_states, dtype=np.float32)
    ws = np.asarray(word_starts).astype(np.int64)
    we = np.asarray(word_ends).astype(np.int64)
    wv = np.asarray(w_attn, dtype=np.float32).reshape(-1)
    ns, hid = H.shape
    nw = ws.shape[0]
    assert hid == HID
    # pre-scale hidden columns by w (bf16); device output is unscaled by host
    wb = wv.astype(ml_dtypes.bfloat16).astype(np.float32)
    assert np.abs(wb).min() > 1e-30
    Hw = (H * wb[None, :]).astype(ml_dtypes.bfloat16)
    Wpc = (nw + NCORES - 1) // NCORES  # words per core
    MT = (Wpc + P - 1) // P  # word-tiles per core
    NG = (MT + 1) // 2  # tile pairs per core

    def tile_bounds(c, m):
        lo = c * Wpc + m * P
        hi = min(lo + P, (c + 1) * Wpc, nw)
        return lo, hi

    def pair_lo(c, g):
        return tile_bounds(c, 2 * g)[0]

    # pair row-block counts and per-tile relative block ranges (max/min over
    # cores so the SPMD program is uniform; masks zero out inactive parts)
    KBp = []
    tile_ks = []
    for g in range(NG):
        kb = 1
        for c in range(NCORES):
            lo = pair_lo(c, g)
            hi = tile_bounds(c, min(2 * g + 1, MT - 1))[1]
            R = int(we[lo:hi].max() - ws[lo] + 1)
            kb = max(kb, (R + P - 1) // P)
        KBp.append(kb)
    for m in range(MT):
        g = m // 2
        k0, k1 = 10**9, -1
        for c in range(NCORES):
            plo = pair_lo(c, g)
            lo, hi = tile_bounds(c, m)
            if lo >= hi:
                continue
            k0 = min(k0, (int(ws[lo]) - int(ws[plo])) // P)
            k1 = max(k1, (int(we[lo:hi].max()) - int(ws[plo])) // P)
        tile_ks.append((k0, k1))
    T = sum(KBp)
    S = sum(k1 - k0 + 1 for k0, k1 in tile_ks)
    base = [0]
    for g in range(NG - 1):
        base.append(base[-1] + KBp[g])

    pos = np.arange(P, dtype=np.int64)
    in_maps = []
    for c in range(NCORES):
        Hg = np.zeros((P, T * HB), ml_dtypes.bfloat16)
        Mk = np.zeros((P, S * P), ml_dtypes.float8_e4m3)
        for g in range(NG):
            kstart = int(ws[pair_lo(c, g)])
            for k in range(KBp[g]):
                j = kstart + k * P + pos  # global rows of this block
                valid = j < ns
                blk = Hg[:, (base[g] + k) * HB : (base[g] + k + 1) * HB]
                blk[valid, :HID] = Hw[j[valid]]
                blk[:, ONES_COL] = 1.0
        s_idx = 0
        for m in range(MT):
            g = m // 2
            kstart = int(ws[pair_lo(c, g)])
            lo, hi = tile_bounds(c, m)
            a = ws[lo:hi] - kstart
            b = we[lo:hi] - kstart
            k0, k1 = tile_ks[m]
            for k in range(k0, k1 + 1):
                jr = (k * P + pos)[:, None]  # rows relative to kstart
                valid = (kstart + k * P + pos) < ns
                msk = (jr >= a[None, :]) & (jr <= b[None, :]) & valid[:, None]
                Mk[:, s_idx * P : s_idx * P + (hi - lo)] = msk.astype(
                    ml_dtypes.float8_e4m3
                )
                s_idx += 1
        in_maps.append({"hg": Hg, "mk": Mk})

    geom = (MT, tuple(KBp), tuple(tile_ks))
    nc = _prog_cache.get(geom)
    if nc is None:
        nc = _build_program(geom)
        _prog_cache[geom] = nc

    res = run_bass_kernel_spmd(nc, in_maps, list(range(NCORES)))
    LAST_RESULT = res
    inv_w = (1.0 / wb).astype(np.float32)
    parts = []
    for c in range(NCORES):
        o = np.asarray(res.results[c]["out"])  # [P, MT*HID] bf16
        o = o.reshape(P, MT, HID).transpose(1, 0, 2).reshape(MT * P, HID)
        parts.append(o[:Wpc])
    full = np.concatenate(parts, axis=0)[:nw].astype(np.float32) * inv_w[None, :]
    return np.ascontiguousarray(full)


# revision 23
# speedup vs baseline: 1.1317x; 1.0291x over previous
"""Word-encoder masked-attention pooling (segment softmax-reduce) on 8 trn2 cores.

Sharding: n_words split across 8 cores (750 words each).  Spans are sorted
and contiguous.  Word tiles (128 words) are packed in PAIRS: both tiles of a
pair share one gathered row window (saves ~17% of row blocks vs per-tile
gathering); a row block straddling both tiles simply feeds two matmul sets.

Host staging (all invertible repacking):
  hg[p, t*1056 + c] : row block t (128 gathered hidden rows, bf16), each
      hidden column c pre-scaled by w_attn[c]; col 1025 = 1.0 (denominator
      matmul operand); 64B-aligned pitch.
  mk[p, s*128 + w]  : 0/1 span mask for matmul-set s (row block x word tile)
Device, per row block t:
  s   = row_sum(hg_block[:, 0:1024])     (= h.w;  DVE reduce / ACT accum)
  e   = Exp(s)                           (ACT, [128,1])
per matmul set (block t, word tile m):
  mke = mask_s * e_t                     (gpsimd tensor_scalar, [128,128])
  pn_m += mke^T @ hg_t[:, 0:1024]        (PE bf16, two 512-wide matmuls)
  pd_m += mke^T @ hg_t[:, 1025]          (PE bf16, 1-wide matmul)
per word tile: r = 1/pd (DVE, 1 tile late), out = pn*r -> bf16 (ACT, 2 tiles
late so it never blocks a later tile in the ACT stream), DMA out.
Host unscales out column c by 1/w_attn[c] and upcasts to f32.
b_attn is constant so it cancels in the softmax.
"""

import ml_dtypes
import numpy as np
from contextlib import ExitStack

import concourse.bass as bass
import concourse.bacc as bacc
import concourse.mybir as mybir
import concourse.tile as tile
from concourse.bass_utils import run_bass_kernel_spmd

NCORES = 8
P = 128
HID = 1024
HB = 1056  # block pitch: 1024 scaled cols + spare + ones col + pad (64B align)
ONES_COL = HID + 1

LAST_RESULT = None  # BassKernelResults of the most recent run (for profiling)

_prog_cache = {}

# row blocks whose score row-sum runs on ACT (Copy+accum) instead of DVE
# (mid-stream: late blocks would clog ACT right when final tiles need it)
ACT_SCORE = (5,)


def _build_program(geom):
    """One SPMD program for all cores.

    geom = (MT, KBp, tile_ks) with KBp[g] row blocks for pair g and
    tile_ks[m] = (k0, k1) the inclusive relative block range tile m consumes.
    """
    MT, KBp, tile_ks = geom
    T = sum(KBp)
    base = [0]
    for g in range(len(KBp) - 1):
        base.append(base[-1] + KBp[g])
    S = sum(k1 - k0 + 1 for k0, k1 in tile_ks)
    f32 = mybir.dt.float32
    bf16 = mybir.dt.bfloat16
    fp8 = mybir.dt.float8e4
    EXP = mybir.ActivationFunctionType.Exp
    COPY = mybir.ActivationFunctionType.Copy
    nc = bacc.Bacc(
        "TRN2", target_bir_lowering=False, debug=False, num_devices=NCORES
    )
    hg = nc.declare_dram_parameter("hg", [P, T * HB], bf16, isOutput=False)
    mk = nc.declare_dram_parameter("mk", [P, S * P], fp8, isOutput=False)
    out = nc.declare_dram_parameter("out", [P, MT * HID], bf16, isOutput=True)

    with tile.TileContext(nc) as tc, ExitStack() as ctx:
        cpool = ctx.enter_context(tc.tile_pool(name="c", bufs=1))
        hpool = ctx.enter_context(tc.tile_pool(name="h", bufs=len(KBp) + 1))
        mpool = ctx.enter_context(tc.tile_pool(name="m", bufs=1))
        mepool = ctx.enter_context(tc.tile_pool(name="me", bufs=6))
        scpool = ctx.enter_context(tc.tile_pool(name="sc", bufs=2))
        spool = ctx.enter_context(tc.tile_pool(name="s", bufs=10))
        rpool = ctx.enter_context(tc.tile_pool(name="r", bufs=3))
        opool = ctx.enter_context(tc.tile_pool(name="o", bufs=3))
        pnpool = ctx.enter_context(tc.tile_pool(name="pn", bufs=3, space="PSUM"))
        pdpool = ctx.enter_context(tc.tile_pool(name="pd", bufs=2, space="PSUM"))

        # warm the ACT exp table off the critical path
        dummy = cpool.tile([P, 1], f32)
        nc.vector.memset(dummy[:], 0.0)
        nc.scalar.activation(dummy[:], dummy[:], EXP)

        # one DMA per h block; block 0 goes on the gpsimd ring, which clears
        # the startup barrier ~0.7us before sync, so the first reduce's data
        # lands earliest; the mask slab follows it (needed later, at first mke)
        htiles = []
        mkt = mpool.tile([P, S * P], fp8)
        issued = 0
        for g in range(len(KBp)):
            ht = hpool.tile([P, KBp[g] * HB], bf16)
            for kb in range(KBp[g]):
                eng = nc.gpsimd if issued == 0 else nc.sync
                eng.dma_start(
                    ht[:, kb * HB : (kb + 1) * HB],
                    hg[:, (base[g] + kb) * HB : (base[g] + kb + 1) * HB],
                )
                issued += 1
                if issued == 1:
                    nc.gpsimd.dma_start(mkt[:], mk[:, :])
            htiles.append(ht)

        def emit_score(t, ht, kb):
            s = spool.tile([P, 1], f32)
            if t in ACT_SCORE:
                scratch = scpool.tile([P, HID], bf16)
                nc.scalar.activation(
                    scratch[:], ht[:, kb * HB : kb * HB + HID], COPY,
                    accum_out=s[:],
                )
            else:
                nc.vector.tensor_reduce(
                    s[:],
                    ht[:, kb * HB : kb * HB + HID],
                    mybir.AxisListType.X,
                    mybir.AluOpType.add,
                )
            e = spool.tile([P, 1], f32)
            nc.scalar.activation(e[:], s[:], EXP)
            return e

        def emit_recip(m, pn, pd):
            r = rpool.tile([P, 1], f32)
            nc.vector.reciprocal(r[:], pd[:, 0:1])
            return r

        def emit_out(m, pn, r, split=False):
            o = opool.tile([P, HID], bf16)
            if split:
                # final tile: halve the serial tail by running the out-scale
                # on ACT and DVE concurrently and the DMA on both rings
                nc.scalar.activation(o[:, 0:512], pn[:, 0:512], COPY, scale=r[:])
                nc.vector.tensor_scalar(
                    out=o[:, 512:HID], in0=pn[:, 512:HID], scalar1=r[:],
                    scalar2=0.0, op0=mybir.AluOpType.mult,
                    op1=mybir.AluOpType.add,
                )
                nc.sync.dma_start(
                    out[:, m * HID : m * HID + 512], o[:, 0:512]
                )
                nc.gpsimd.dma_start(
                    out[:, m * HID + 512 : (m + 1) * HID], o[:, 512:HID]
                )
                return
            nc.scalar.activation(o[:], pn[:], COPY, scale=r[:])
            eng = nc.sync if m % 2 == 0 else nc.gpsimd
            eng.dma_start(out[:, m * HID : (m + 1) * HID], o[:])

        s_idx = 0
        q = []  # [m, pn, pd, r] tails awaiting emission, oldest first
        for g in range(len(KBp)):
            ht = htiles[g]
            evals = [emit_score(base[g] + kb, ht, kb) for kb in range(KBp[g])]
            for m in (2 * g, 2 * g + 1):
                if m >= MT:
                    continue
                k0, k1 = tile_ks[m]
                pn = pnpool.tile([P, HID], f32)
                pd = pdpool.tile([P, 8], f32)
                for kb in range(k0, k1 + 1):
                    mke = mepool.tile([P, P], bf16)
                    nc.gpsimd.tensor_scalar(
                        out=mke[:],
                        in0=mkt[:, s_idx * P : (s_idx + 1) * P],
                        scalar1=evals[kb][:],
                        scalar2=0.0,
                        op0=mybir.AluOpType.mult,
                        op1=mybir.AluOpType.add,
                    )
                    s_idx += 1
                    first, last = kb == k0, kb == k1
                    nc.tensor.matmul(
                        pd[:, 0:1],
                        mke[:],
                        ht[:, kb * HB + ONES_COL : kb * HB + ONES_COL + 1],
                        start=first,
                        stop=last,
                    )
                    for half in range(2):
                        cs = slice(
                            kb * HB + half * 512, kb * HB + (half + 1) * 512
                        )
                        nc.tensor.matmul(
                            pn[:, half * 512 : (half + 1) * 512],
                            mke[:],
                            ht[:, cs],
                            start=first,
                            stop=last,
                        )

                q.append([m, pn, pd, None])
                if len(q) >= 2:
                    q[-2][3] = emit_recip(q[-2][0], q[-2][1], q[-2][2])
                if len(q) >= 3:
                    mm, pnn, _, rr = q.pop(0)
                    emit_out(mm, pnn, rr)
        q[-1][3] = emit_recip(q[-1][0], q[-1][1], q[-1][2])
        for i, (mm, pnn, _, rr) in enumerate(q):
            emit_out(mm, pnn, rr, split=(i == len(q) - 1))

    nc.compile()
    return nc


def kernel(hidden_states, word_starts, word_ends, w_attn, b_attn):
    global LAST_RESULT
    H = np.asarray(hidden_states, dtype=np.float32)
    ws = np.asarray(word_starts).astype(np.int64)
    we = np.asarray(word_ends).astype(np.int64)
    wv = np.asarray(w_attn, dtype=np.float32).reshape(-1)
    ns, hid = H.shape
    nw = ws.shape[0]
    assert hid == HID
    # pre-scale hidden columns by w (bf16); device output is unscaled by host
    wb = wv.astype(ml_dtypes.bfloat16).astype(np.float32)
    assert np.abs(wb).min() > 1e-30
    Hw = (H * wb[None, :]).astype(ml_dtypes.bfloat16)
    Wpc = (nw + NCORES - 1) // NCORES  # words per core
    MT = (Wpc + P - 1) // P  # word-tiles per core
    NG = (MT + 1) // 2  # tile pairs per core

    def tile_bounds(c, m):
        lo = c * Wpc + m * P
        hi = min(lo + P, (c + 1) * Wpc, nw)
        return lo, hi

    def pair_lo(c, g):
        return tile_bounds(c, 2 * g)[0]

    # pair row-block counts and per-tile relative block ranges (max/min over
    # cores so the SPMD program is uniform; masks zero out inactive parts)
    KBp = []
    tile_ks = []
    for g in range(NG):
        kb = 1
        for c in range(NCORES):
            lo = pair_lo(c, g)
            hi = tile_bounds(c, min(2 * g + 1, MT - 1))[1]
            R = int(we[lo:hi].max() - ws[lo] + 1)
            kb = max(kb, (R + P - 1) // P)
        KBp.append(kb)
    for m in range(MT):
        g = m // 2
        k0, k1 = 10**9, -1
        for c in range(NCORES):
            plo = pair_lo(c, g)
            lo, hi = tile_bounds(c, m)
            if lo >= hi:
                continue
            k0 = min(k0, (int(ws[lo]) - int(ws[plo])) // P)
            k1 = max(k1, (int(we[lo:hi].max()) - int(ws[plo])) // P)
        tile_ks.append((k0, k1))
    T = sum(KBp)
    S = sum(k1 - k0 + 1 for k0, k1 in tile_ks)
    base = [0]
    for g in range(NG - 1):
        base.append(base[-1] + KBp[g])

    pos = np.arange(P, dtype=np.int64)
    in_maps = []
    for c in range(NCORES):
        Hg = np.zeros((P, T * HB), ml_dtypes.bfloat16)
        Mk = np.zeros((P, S * P), ml_dtypes.float8_e4m3)
        for g in range(NG):
            kstart = int(ws[pair_lo(c, g)])
            for k in range(KBp[g]):
                j = kstart + k * P + pos  # global rows of this block
                valid = j < ns
                blk = Hg[:, (base[g] + k) * HB : (base[g] + k + 1) * HB]
                blk[valid, :HID] = Hw[j[valid]]
                blk[:, ONES_COL] = 1.0
        s_idx = 0
        for m in range(MT):
            g = m // 2
            kstart = int(ws[pair_lo(c, g)])
            lo, hi = tile_bounds(c, m)
            a = ws[lo:hi] - kstart
            b = we[lo:hi] - kstart
            k0, k1 = tile_ks[m]
            for k in range(k0, k1 + 1):
                jr = (k * P + pos)[:, None]  # rows relative to kstart
                valid = (kstart + k * P + pos) < ns
                msk = (jr >= a[None, :]) & (jr <= b[None, :]) & valid[:, None]
                Mk[:, s_idx * P : s_idx * P + (hi - lo)] = msk.astype(
                    ml_dtypes.float8_e4m3
                )
                s_idx += 1
        in_maps.append({"hg": Hg, "mk": Mk})

    geom = (MT, tuple(KBp), tuple(tile_ks))
    nc = _prog_cache.get(geom)
    if nc is None:
        nc = _build_program(geom)
        _prog_cache[geom] = nc

    res = run_bass_kernel_spmd(nc, in_maps, list(range(NCORES)))
    LAST_RESULT = res
    inv_w = (1.0 / wb).astype(np.float32)
    parts = []
    for c in range(NCORES):
        o = np.asarray(res.results[c]["out"])  # [P, MT*HID] bf16
        o = o.reshape(P, MT, HID).transpose(1, 0, 2).reshape(MT * P, HID)
        parts.append(o[:Wpc])
    full = np.concatenate(parts, axis=0)[:nw].astype(np.float32) * inv_w[None, :]
    return np.ascontiguousarray(full)
